# revision 2
# baseline (speedup 1.0000x reference)
"""Distributed multi-head attention for Trainium2 (8 NeuronCores).

Problem: B=2, T=4096, E=128, H=8 dense attention
    keys/queries/values = x @ W{k,q,v}      [b, t, 1024] -> heads
    att = softmax(Q K^T / sqrt(E)); out = (att V) @ Wu

Sharding (hardcoded): core c handles batch b = c // 4 and global heads
{2g, 2g+1} with g = c % 4 — data parallel on batch, tensor parallel on
heads.  Each core computes its two heads' attention plus the
head-sliced unifyheads matmul, giving a partial [E, T] output (stored
transposed); an in-group AllReduce over {0..3} / {4..7} produces the
full per-batch output on every core of the group.

Device layout notes:
  * All big matmuls contract over the partition axis.  Inputs are fed
    pre-transposed ([E, T] "xT") so projections produce queries^T /
    keys^T directly; scores are computed transposed (S^T [k, q]) so the
    softmax'd P^T tiles feed the A@V matmul with no on-chip transposes.
  * Scores run in float32r (full-rate fp32 PE mode); P/V/Wu in bf16.
  * Softmax denominators: DVE accumulates P^T tiles (bf16) over k-tiles,
    an all-ones bf16 matmul reduces over partitions into PSUM (f32) and
    broadcasts to 128 partitions; 1/s = exp(-ln(s)) on ScalarE (both
    functions live in one ACT table set).  Max-subtraction is skipped:
    logits are provably in [-3, 3] for this problem's input scaling.
"""

import numpy as np
import ml_dtypes

import concourse.bass as bass
import concourse.bacc as bacc
import concourse.tile as tile
import concourse.mybir as mybir
from concourse.bass_utils import run_bass_kernel_spmd

B = 2
T = 4096
E = 128
H = 8
P = 128
N_CORES = 8
QC = 1024          # q-chunk width (columns per PSUM scores tile)
NQC = T // QC      # 4 q-chunks
NK = T // P        # 32 k-tiles
NT = T // P        # 32 t-tiles (values projection)
SCALE = float(1.0 / np.sqrt(np.float32(E)))
GROUPS = [[0, 1, 2, 3], [4, 5, 6, 7]]

F32 = mybir.dt.float32
F32R = mybir.dt.float32r
BF16 = mybir.dt.bfloat16
EXP = mybir.ActivationFunctionType.Exp
LN = mybir.ActivationFunctionType.Ln
ADD = mybir.AluOpType.add
MULT = mybir.AluOpType.mult

TRACE = False
LAST_EXEC_NS = None
_CACHE = {}


def _build():
    nc = bacc.Bacc(None, target_bir_lowering=False)
    kT_e = nc.declare_dram_parameter("kT", [P, T], F32R, isOutput=False)
    qT_e = nc.declare_dram_parameter("qT", [P, T], F32R, isOutput=False)
    vT_e = nc.declare_dram_parameter("vT", [P, T], F32R, isOutput=False)
    wk_e = nc.declare_dram_parameter("wk", [P, 256], F32R, isOutput=False)
    wq_e = nc.declare_dram_parameter("wq", [P, 256], F32R, isOutput=False)
    wv_e = nc.declare_dram_parameter("wv", [P, 256], F32R, isOutput=False)
    wu_e = nc.declare_dram_parameter("wu", [256, E], BF16, isOutput=False)
    ones_e = nc.declare_dram_parameter("ones", [P, P], BF16, isOutput=False)
    out_e = nc.declare_dram_parameter("out", [P, T], F32, isOutput=True)

    with tile.TileContext(nc) as tc:
        with (
            tc.tile_pool(name="const", bufs=1) as constp,
            tc.tile_pool(name="xt", bufs=1) as xtp,
            tc.tile_pool(name="proj", bufs=1) as projp,
            tc.tile_pool(name="pp", bufs=6) as ppool,
            tc.tile_pool(name="accp", bufs=2) as accp,
            tc.tile_pool(name="small", bufs=2) as smallp,
            tc.tile_pool(name="outh", bufs=2) as outhp,
            tc.tile_pool(name="scp", bufs=2, space="PSUM") as scp,
            tc.tile_pool(name="avp", bufs=1, space="PSUM") as avp,
            tc.tile_pool(name="auxp", bufs=1, space="PSUM") as auxp,
            tc.tile_pool(name="dram", bufs=1, space="DRAM") as dramp,
        ):
            # ---- constants & inputs -------------------------------------
            wk_s = constp.tile([P, 256], F32R, tag="wk")
            wq_s = constp.tile([P, 256], F32R, tag="wq")
            wv_s = constp.tile([P, 256], F32R, tag="wv")
            wu_s = constp.tile([P, 256], BF16, tag="wu")
            ones_s = constp.tile([P, P], BF16, tag="ones")
            nc.sync.dma_start(out=wk_s[:], in_=wk_e[:, :])
            nc.sync.dma_start(out=wq_s[:], in_=wq_e[:, :])
            nc.sync.dma_start(out=wv_s[:], in_=wv_e[:, :])
            for h in range(2):
                nc.sync.dma_start(
                    out=wu_s[:, h * E:(h + 1) * E],
                    in_=wu_e[h * E:(h + 1) * E, :],
                )
            nc.sync.dma_start(out=ones_s[:], in_=ones_e[:, :])

            kT_s = xtp.tile([P, T], F32R, tag="kT")
            qT_s = xtp.tile([P, T], F32R, tag="qT")
            vT_s = xtp.tile([P, T], F32R, tag="vT")
            nc.sync.dma_start(out=kT_s[:], in_=kT_e[:, :])
            nc.sync.dma_start(out=qT_s[:], in_=qT_e[:, :])
            nc.sync.dma_start(out=vT_s[:], in_=vT_e[:, :])

            # ---- projections --------------------------------------------
            # queries^T / keys^T per head: [E, T] f32r
            qh = [projp.tile([P, T], F32R, tag=f"qh{h}", name=f"qh{h}") for h in range(2)]
            kh = [projp.tile([P, T], F32R, tag=f"kh{h}", name=f"kh{h}") for h in range(2)]
            for h in range(2):
                for ch in range(8):
                    sl = slice(ch * 512, (ch + 1) * 512)
                    ps = scp.tile([P, QC], F32, tag="sc")
                    nc.tensor.matmul(
                        ps[:, 0:512], wq_s[:, h * E:(h + 1) * E],
                        qT_s[:, sl], start=True, stop=True,
                    )
                    nc.scalar.activation(
                        qh[h][:, sl], ps[:, 0:512],
                        mybir.ActivationFunctionType.Copy,
                    )
                    ps2 = scp.tile([P, QC], F32, tag="sc")
                    nc.tensor.matmul(
                        ps2[:, 0:512], wk_s[:, h * E:(h + 1) * E],
                        kT_s[:, sl], start=True, stop=True,
                    )
                    nc.vector.tensor_copy(kh[h][:, sl], ps2[:, 0:512])

            # values [t, e'] bf16, tile-major: [128, tt*256 + e']
            vals = projp.tile([P, NT * 256], BF16, tag="vals")
            for tt in range(NT):
                ps = scp.tile([P, QC], F32, tag="sc")
                nc.tensor.matmul(
                    ps[:, 0:256], vT_s[:, tt * P:(tt + 1) * P],
                    wv_s[:], start=True, stop=True,
                )
                nc.vector.tensor_copy(
                    vals[:, tt * 256:(tt + 1) * 256], ps[:, 0:256]
                )

            # ---- attention ----------------------------------------------
            partial = dramp.tile([P, T], F32, tag="partial")
            for qc in range(NQC):
                q0 = qc * QC
                oh_tiles = []
                for h in range(2):
                    acc_lo = accp.tile([P, QC], BF16, tag="acclo")
                    acc_hi = accp.tile([P, QC], BF16, tag="acchi")
                    av = avp.tile([P, QC], F32, tag="av")
                    for kk in range(NK):
                        ksl = kh[h][:, kk * P:(kk + 1) * P]
                        sc = scp.tile([P, QC], F32, tag="sc")
                        nc.tensor.matmul(
                            sc[:, 0:512], ksl, qh[h][:, q0:q0 + 512],
                            start=True, stop=True,
                        )
                        nc.tensor.matmul(
                            sc[:, 512:QC], ksl, qh[h][:, q0 + 512:q0 + QC],
                            start=True, stop=True,
                        )
                        p = ppool.tile([P, QC], BF16, tag="p")
                        nc.scalar.activation(p[:], sc[:], EXP, scale=SCALE)
                        if kk == 0:
                            nc.vector.tensor_copy(acc_lo[:], p[:])
                        elif kk == 16:
                            nc.vector.tensor_copy(acc_hi[:], p[:])
                        else:
                            acc = acc_lo if kk < 16 else acc_hi
                            nc.vector.tensor_tensor(
                                out=acc[:], in0=acc[:], in1=p[:], op=ADD
                            )
                        vsl = vals[:, kk * 256 + h * E: kk * 256 + (h + 1) * E]
                        nc.tensor.matmul(
                            av[:, 0:512], vsl, p[:, 0:512],
                            start=(kk == 0), stop=(kk == NK - 1),
                        )
                        nc.tensor.matmul(
                            av[:, 512:QC], vsl, p[:, 512:QC],
                            start=(kk == 0), stop=(kk == NK - 1),
                        )
                    # softmax denominators, broadcast to all partitions
                    sums = auxp.tile([P, QC], F32, tag="aux")
                    for half in range(2):
                        hsl = slice(half * 512, (half + 1) * 512)
                        nc.tensor.matmul(
                            sums[:, hsl], ones_s[:], acc_lo[:, hsl],
                            start=True, stop=False,
                        )
                        nc.tensor.matmul(
                            sums[:, hsl], ones_s[:], acc_hi[:, hsl],
                            start=False, stop=True,
                        )
                    lns = smallp.tile([P, QC], F32, tag="lns")
                    nc.scalar.activation(lns[:], sums[:], LN)
                    r = smallp.tile([P, QC], F32, tag="r")
                    nc.scalar.activation(r[:], lns[:], EXP, scale=-1.0)
                    oh = outhp.tile([P, QC], BF16, tag=f"oh{h}")
                    nc.vector.tensor_tensor(
                        out=oh[:], in0=av[:], in1=r[:], op=MULT
                    )
                    oh_tiles.append(oh)
                # unifyheads: accumulate both heads into one PSUM tile
                u = auxp.tile([P, QC], F32, tag="aux")
                for h in range(2):
                    for half in range(2):
                        hsl = slice(half * 512, (half + 1) * 512)
                        nc.tensor.matmul(
                            u[:, hsl], wu_s[:, h * E:(h + 1) * E],
                            oh_tiles[h][:, hsl],
                            start=(h == 0), stop=(h == 1),
                        )
                us = smallp.tile([P, QC], F32, tag="us")
                nc.vector.tensor_copy(us[:], u[:])
                nc.sync.dma_start(out=partial[:, q0:q0 + QC], in_=us[:])

            # ---- in-group AllReduce over the head-parallel cores --------
            outb = dramp.tile([P, T], F32, tag="outb")
            nc.gpsimd.collective_compute(
                "AllReduce",
                ADD,
                replica_groups=GROUPS,
                ins=[partial.opt()],
                outs=[outb.opt()],
            )
            nc.sync.dma_start(out=out_e[:, :], in_=outb[:])
    nc.finalize()
    return nc


def _get_nc():
    if "nc" not in _CACHE:
        _CACHE["nc"] = _build()
    return _CACHE["nc"]


def kernel(k, q, v, Wk, Wq, Wv, Wu):
    global LAST_EXEC_NS
    k = np.asarray(k, np.float32)
    q = np.asarray(q, np.float32)
    v = np.asarray(v, np.float32)
    Wk = np.asarray(Wk, np.float32)
    Wq = np.asarray(Wq, np.float32)
    Wv = np.asarray(Wv, np.float32)
    Wu = np.asarray(Wu, np.float32)

    ones = np.ones((P, P), dtype=ml_dtypes.bfloat16)
    in_maps = []
    xT = {}
    for b in range(B):
        xT[b] = (
            np.ascontiguousarray(k[b].T),
            np.ascontiguousarray(q[b].T),
            np.ascontiguousarray(v[b].T),
        )
    for c in range(N_CORES):
        b, g = c // 4, c % 4
        cols = slice(g * 256, (g + 1) * 256)
        in_maps.append({
            "kT": xT[b][0],
            "qT": xT[b][1],
            "vT": xT[b][2],
            "wk": np.ascontiguousarray(Wk[:, cols]),
            "wq": np.ascontiguousarray(Wq[:, cols]),
            "wv": np.ascontiguousarray(Wv[:, cols]),
            "wu": np.ascontiguousarray(Wu[cols, :]).astype(ml_dtypes.bfloat16),
            "ones": ones,
        })

    nc = _get_nc()
    res = run_bass_kernel_spmd(
        nc, in_maps, core_ids=list(range(N_CORES)), trace=TRACE
    )
    LAST_EXEC_NS = res.exec_time_ns
    out = np.stack(
        [res.results[0]["out"].T, res.results[4]["out"].T]
    )  # [B, T, E]
    return np.ascontiguousarray(out.astype(np.float32))


# revision 5
# speedup vs baseline: 1.2062x; 1.2062x over previous
"""Distributed multi-head attention for Trainium2 (8 NeuronCores).

Problem: B=2, T=4096, E=128, H=8 dense attention
    keys/queries/values = x @ W{k,q,v}      [b, t, 1024] -> heads
    att = softmax(Q K^T / sqrt(E)); out = (att V) @ Wu

Sharding (hardcoded): core c handles batch b = c // 4 and global heads
{2g, 2g+1} with g = c % 4 — data parallel on batch, tensor parallel on
heads.  Each core computes its two heads' attention plus the
head-sliced unifyheads matmul, giving a partial [E, T] output (stored
transposed); in-group ReduceScatters over {0..3} / {4..7} leave each
core with a 32-row shard of the per-batch output, reassembled on host.

Device layout notes:
  * All big matmuls contract over the partition axis.  Inputs are fed
    pre-transposed ([E, T] "xT") so projections produce queries^T /
    keys^T directly; scores are computed transposed (S^T [k, q]) so the
    softmax'd P^T tiles feed the A@V matmul with no on-chip transposes.
  * Projections run in float32r (full-rate fp32 PE mode); the attention
    matmuls in bf16 (1024-wide moving operands, FWL weight loads).
  * Softmax denominators: DVE accumulates P^T tiles (bf16) over k-tiles,
    an all-ones bf16 matmul reduces over partitions into PSUM (f32) and
    broadcasts to 128 partitions; 1/s = exp(-ln(s)) on ScalarE.  Both
    functions come from one ACT table set (see _patched_tables).
    Max-subtraction is skipped: logits are provably within ~[-3, 3] for
    this problem's input scaling, so exp cannot overflow.
"""

import numpy as np
import ml_dtypes

import concourse.bass as bass
import concourse.bacc as bacc
import concourse.tile as tile
import concourse.mybir as mybir
from concourse.bass_utils import run_bass_kernel_spmd

B = 2
T = 4096
E = 128
H = 8
P = 128
N_CORES = 8
QC = 1024          # q-chunk width (columns per PSUM scores tile)
NQC = T // QC      # 4 q-chunks
NK = T // P        # 32 k-tiles
NT = T // P        # 32 t-tiles (values projection)
SCALE = float(1.0 / np.sqrt(np.float32(E)))
GROUPS = [[0, 1, 2, 3], [4, 5, 6, 7]]

F32 = mybir.dt.float32
F32R = mybir.dt.float32r
BF16 = mybir.dt.bfloat16
EXP = mybir.ActivationFunctionType.Exp
LN = mybir.ActivationFunctionType.Ln
COPY = mybir.ActivationFunctionType.Copy
ADD = mybir.AluOpType.add
MULT = mybir.AluOpType.mult

TRACE = False
LAST_EXEC_NS = None
_CACHE = {}


def _patched_tables(arch):
    """Only let the act-table chooser see Exp/Ln in the one set that has
    both, so the per-chunk Ln doesn't thrash table reloads (~2.7us each).
    Set indices (= act_func_set_id) are preserved."""
    tabs = _CACHE["orig_tables"](arch)
    out = {}
    for name, fns in tabs.items():
        if name != "natural_log_exp_and_others":
            fns = {f for f in fns
                   if f not in (EXP, LN)}
        out[name] = fns
    return out


def _build():
    _CACHE.setdefault("orig_tables", bacc.get_activation_tables)
    bacc.get_activation_tables = _patched_tables

    nc = bacc.Bacc(None, target_bir_lowering=False)
    kT_e = nc.declare_dram_parameter("kT", [P, T], F32R, isOutput=False)
    qT_e = nc.declare_dram_parameter("qT", [P, T], F32R, isOutput=False)
    vT_e = nc.declare_dram_parameter("vT", [P, T], F32R, isOutput=False)
    wk_e = nc.declare_dram_parameter("wk", [P, 256], F32R, isOutput=False)
    wq_e = nc.declare_dram_parameter("wq", [P, 256], F32R, isOutput=False)
    wv_e = nc.declare_dram_parameter("wv", [P, 256], F32R, isOutput=False)
    wu_e = nc.declare_dram_parameter("wu", [256, E], BF16, isOutput=False)
    ones_e = nc.declare_dram_parameter("ones", [P, P], BF16, isOutput=False)
    out_e = nc.declare_dram_parameter("out", [32, T], F32, isOutput=True)

    with tile.TileContext(nc) as tc:
        with (
            tc.tile_pool(name="const", bufs=1) as constp,
            tc.tile_pool(name="xt", bufs=1) as xtp,
            tc.tile_pool(name="proj", bufs=1) as projp,
            tc.tile_pool(name="pp", bufs=6) as ppool,
            tc.tile_pool(name="accp", bufs=2) as accp,
            tc.tile_pool(name="small", bufs=2) as smallp,
            tc.tile_pool(name="outh", bufs=2) as outhp,
            tc.tile_pool(name="scp", bufs=2, space="PSUM") as scp,
            tc.tile_pool(name="avp", bufs=1, space="PSUM") as avp,
            tc.tile_pool(name="auxp", bufs=1, space="PSUM") as auxp,
            tc.tile_pool(name="dram", bufs=1, space="DRAM") as dramp,
        ):
            # ---- constants & inputs -------------------------------------
            wk_s = constp.tile([P, 256], F32R, tag="wk")
            wq_s = constp.tile([P, 256], F32R, tag="wq")
            wv_s = constp.tile([P, 256], F32R, tag="wv")
            wu_s = constp.tile([P, 256], BF16, tag="wu")
            ones_s = constp.tile([P, P], BF16, tag="ones")
            nc.sync.dma_start(out=wk_s[:], in_=wk_e[:, :])
            nc.sync.dma_start(out=wq_s[:], in_=wq_e[:, :])
            nc.sync.dma_start(out=wv_s[:], in_=wv_e[:, :])
            for h in range(2):
                nc.sync.dma_start(
                    out=wu_s[:, h * E:(h + 1) * E],
                    in_=wu_e[h * E:(h + 1) * E, :],
                )
            nc.sync.dma_start(out=ones_s[:], in_=ones_e[:, :])

            # chunked loads so projections can start early
            kT_s = xtp.tile([P, T], F32R, tag="kT")
            qT_s = xtp.tile([P, T], F32R, tag="qT")
            vT_s = xtp.tile([P, T], F32R, tag="vT")
            for t, e in ((qT_s, qT_e), (kT_s, kT_e), (vT_s, vT_e)):
                for ch in range(4):
                    sl = slice(ch * QC, (ch + 1) * QC)
                    nc.sync.dma_start(out=t[:, sl], in_=e[:, sl])

            # ---- projections (f32r), outputs cast to bf16 ---------------
            qh = [projp.tile([P, T], BF16, tag=f"qh{h}", name=f"qh{h}")
                  for h in range(2)]
            kh = [projp.tile([P, T], BF16, tag=f"kh{h}", name=f"kh{h}")
                  for h in range(2)]
            for h in range(2):
                for ch in range(8):
                    sl = slice(ch * 512, (ch + 1) * 512)
                    ps = scp.tile([P, QC], F32, tag="sc")
                    nc.tensor.matmul(
                        ps[:, 0:512], wq_s[:, h * E:(h + 1) * E],
                        qT_s[:, sl], start=True, stop=True,
                    )
                    nc.scalar.activation(qh[h][:, sl], ps[:, 0:512], COPY)
                    ps2 = scp.tile([P, QC], F32, tag="sc")
                    nc.tensor.matmul(
                        ps2[:, 0:512], wk_s[:, h * E:(h + 1) * E],
                        kT_s[:, sl], start=True, stop=True,
                    )
                    nc.vector.tensor_copy(kh[h][:, sl], ps2[:, 0:512])

            # values [t, e'] bf16, tile-major: [128, tt*256 + e']
            vals = projp.tile([P, NT * 256], BF16, tag="vals")
            for tt in range(NT):
                ps = scp.tile([P, QC], F32, tag="sc")
                nc.tensor.matmul(
                    ps[:, 0:256], vT_s[:, tt * P:(tt + 1) * P],
                    wv_s[:], start=True, stop=True,
                )
                nc.vector.tensor_copy(
                    vals[:, tt * 256:(tt + 1) * 256], ps[:, 0:256]
                )

            # ---- attention ----------------------------------------------
            halves = [
                dramp.tile([P, T // 2], F32, tag=f"partial{i}",
                           name=f"partial{i}")
                for i in range(2)
            ]
            rs_outs = [
                dramp.tile([32, T // 2], F32, tag=f"rs{i}", name=f"rs{i}")
                for i in range(2)
            ]
            for qc in range(NQC):
                q0 = qc * QC
                oh_tiles = []
                for h in range(2):
                    acc_lo = accp.tile([P, QC], BF16, tag="acclo")
                    acc_hi = accp.tile([P, QC], BF16, tag="acchi")
                    av = avp.tile([P, QC], F32, tag="av")
                    for kk in range(NK):
                        ksl = kh[h][:, kk * P:(kk + 1) * P]
                        sc = scp.tile([P, QC], F32, tag="sc")
                        nc.tensor.matmul(
                            sc[:, 0:512], ksl, qh[h][:, q0:q0 + 512],
                            start=True, stop=True,
                        )
                        nc.tensor.matmul(
                            sc[:, 512:QC], ksl, qh[h][:, q0 + 512:q0 + QC],
                            start=True, stop=True,
                        )
                        p = ppool.tile([P, QC], BF16, tag="p")
                        nc.scalar.activation(p[:], sc[:], EXP, scale=SCALE)
                        if kk == 0:
                            nc.vector.tensor_copy(acc_lo[:], p[:])
                        elif kk == 16:
                            nc.vector.tensor_copy(acc_hi[:], p[:])
                        else:
                            acc = acc_lo if kk < 16 else acc_hi
                            nc.vector.tensor_tensor(
                                out=acc[:], in0=acc[:], in1=p[:], op=ADD
                            )
                        vsl = vals[:, kk * 256 + h * E: kk * 256 + (h + 1) * E]
                        nc.tensor.matmul(
                            av[:, 0:512], vsl, p[:, 0:512],
                            start=(kk == 0), stop=(kk == NK - 1),
                        )
                        nc.tensor.matmul(
                            av[:, 512:QC], vsl, p[:, 512:QC],
                            start=(kk == 0), stop=(kk == NK - 1),
                        )
                    # softmax denominators, broadcast to all partitions
                    sums = auxp.tile([P, QC], F32, tag="aux")
                    for half in range(2):
                        hsl = slice(half * 512, (half + 1) * 512)
                        nc.tensor.matmul(sums[:, hsl], ones_s[:],
                                         acc_lo[:, hsl],
                                         start=True, stop=False)
                        nc.tensor.matmul(sums[:, hsl], ones_s[:],
                                         acc_hi[:, hsl],
                                         start=False, stop=True)
                    lns = smallp.tile([P, QC], F32, tag="lns")
                    nc.scalar.activation(lns[:], sums[:], LN)
                    r = smallp.tile([P, QC], F32, tag="r")
                    nc.scalar.activation(r[:], lns[:], EXP, scale=-1.0)
                    oh = outhp.tile([P, QC], BF16, tag=f"oh{h}")
                    nc.vector.tensor_tensor(
                        out=oh[:], in0=av[:], in1=r[:], op=MULT
                    )
                    oh_tiles.append(oh)
                # unifyheads: accumulate both heads into one PSUM tile
                u = auxp.tile([P, QC], F32, tag="aux")
                for h in range(2):
                    for half in range(2):
                        hsl = slice(half * 512, (half + 1) * 512)
                        nc.tensor.matmul(
                            u[:, hsl], wu_s[:, h * E:(h + 1) * E],
                            oh_tiles[h][:, hsl],
                            start=(h == 0), stop=(h == 1),
                        )
                us = smallp.tile([P, QC], F32, tag="us")
                nc.vector.tensor_copy(us[:], u[:])
                nc.sync.dma_start(
                    out=halves[qc // 2][:, (qc % 2) * QC:(qc % 2 + 1) * QC],
                    in_=us[:],
                )
                if qc % 2 == 1:
                    # ReduceScatter this half; the first one overlaps the
                    # second half's compute
                    nc.gpsimd.collective_compute(
                        "ReduceScatter",
                        ADD,
                        replica_groups=GROUPS,
                        ins=[halves[qc // 2].opt()],
                        outs=[rs_outs[qc // 2].opt()],
                    )
                    nc.sync.dma_start(
                        out=out_e[:, (qc // 2) * (T // 2):
                                  (qc // 2 + 1) * (T // 2)],
                        in_=rs_outs[qc // 2][:],
                    )
    nc.finalize()
    bacc.get_activation_tables = _CACHE["orig_tables"]
    return nc


def _get_nc():
    if "nc" not in _CACHE:
        _CACHE["nc"] = _build()
    return _CACHE["nc"]


def kernel(k, q, v, Wk, Wq, Wv, Wu):
    global LAST_EXEC_NS
    k = np.asarray(k, np.float32)
    q = np.asarray(q, np.float32)
    v = np.asarray(v, np.float32)
    Wk = np.asarray(Wk, np.float32)
    Wq = np.asarray(Wq, np.float32)
    Wv = np.asarray(Wv, np.float32)
    Wu = np.asarray(Wu, np.float32)

    ones = np.ones((P, P), dtype=ml_dtypes.bfloat16)
    in_maps = []
    xT = {}
    for b in range(B):
        xT[b] = (
            np.ascontiguousarray(k[b].T),
            np.ascontiguousarray(q[b].T),
            np.ascontiguousarray(v[b].T),
        )
    for c in range(N_CORES):
        b, g = c // 4, c % 4
        cols = slice(g * 256, (g + 1) * 256)
        in_maps.append({
            "kT": xT[b][0],
            "qT": xT[b][1],
            "vT": xT[b][2],
            "wk": np.ascontiguousarray(Wk[:, cols]),
            "wq": np.ascontiguousarray(Wq[:, cols]),
            "wv": np.ascontiguousarray(Wv[:, cols]),
            "wu": np.ascontiguousarray(Wu[cols, :]).astype(ml_dtypes.bfloat16),
            "ones": ones,
        })

    nc = _get_nc()
    res = run_bass_kernel_spmd(
        nc, in_maps, core_ids=list(range(N_CORES)), trace=TRACE
    )
    LAST_EXEC_NS = res.exec_time_ns
    # each group core holds a 32-row shard of the batch's [E, T] output
    out = np.empty((B, T, E), np.float32)
    for b in range(B):
        outT = np.concatenate(
            [res.results[4 * b + r]["out"] for r in range(4)], axis=0
        )  # [128, T]
        out[b] = outT.T
    return out
